# revision 33
# baseline (speedup 1.0000x reference)
"""AdaptivePruner Trainium2 kernel: gini-routed 1/2-level db4 DWT lowpass.

Strategy
--------
- Routing (gini > SOGLIA) is computed on host with jax-on-CPU, bit-matching
  the reference's float32 arithmetic (one row's gini sits 4e-7 from the
  threshold, so arithmetic-order fidelity matters).
- The DWT is one dense matmul per sample: out[b] = Cx_b.T @ x[b] where
  Cx_b (197x102) is the host-selected composite matrix (level-1 conv, or
  level-1 o level-2 conv zero-padded, with an identity entry passing the
  cls token through). Mask is reconstructed on host.
- Pure data parallelism: batch 256 -> 8 NeuronCores x 32 samples.
- DMA shape rules (measured on this part): a dma_start fans across all 16
  SDMA engines only at 128 partitions, and per-engine throughput is
  bounded by max(bytes/25.6 GB/s, ~118 ns/descriptor). All DRAM tensors
  are laid out partition-major (k, b, ...) and moved in 4-batch groups:
  128 descriptors of 4x-size per group.
- Contraction split 128+128 with a 59-token overlap (overlap rows zeroed
  in the weight chunk B); host interleaves both chunks per partition row.
- x/weights/output all ride bf16 (host converts and upcasts): full PE
  rate, fast weight load, half the DMA bytes. M is padded to 128 so the
  padded output rows are exact zeros and every DMA is 128-partition.
- DMA issue alternates between the two HWDGE rings (SP and Activation)
  per group; PSUM->SBUF cast-copies alternate Vector/Scalar engines.
"""

import os
import sys

import numpy as np

for _p in ("/opt/trn_rl_repo", "/root/.axon_site/_ro/trn_rl_repo"):
    if os.path.isdir(_p) and _p not in sys.path:
        sys.path.append(_p)

import concourse.bacc as bacc
import concourse.mybir as mybir
from concourse.tile import TileContext
from concourse.bass_utils import run_bass_kernel_spmd

SOGLIA = 0.333
DB4_H = np.array([0.23037781330885523, 0.7148465705525415, 0.6308807679295904,
                  -0.02798376941698385, -0.18703481171888114, 0.030841381835986965,
                  0.032883011666982945, -0.010597401784997278], dtype=np.float32)

B, N_TOK, D = 256, 197, 768          # x: (B, 197, 768)
NP_ = 196                            # patch tokens
LEN1, LEN2 = 101, 54                 # dwt output lengths
OUT_TOK = LEN1 + 1                   # 102 = cls + padded patches
M_PAD = 128                          # matmul M / padded output rows
N_CORES = 8
B_LOC = 32                           # batches per core
G = 4                                # batches per DMA group
KA = 128                             # chunk A: tokens 0..128
KB_OFF = N_TOK - 128                 # 69: chunk B covers tokens 69..197
XW_COLS = 2 * D                      # interleaved input row: [chunk A | chunk B]
W_COLS = 2 * M_PAD                   # merged weight row: [chunk A | chunk B]
LAST_RESULT = None                   # BassKernelResults of the last run
RUN_KWARGS = {}                      # extra kwargs for run_bass_kernel_spmd (test harness)
_NC_CACHE = []                       # compiled Bacc program, built once per process


def _conv_mats():
    """Composite DWT matrices in float64: Cx1/Cx2 (197, 102).

    Row 0 / col 0 pass the cls token through; rows 1+k / cols 1+t hold the
    level-1 (C1) or level-1 compose level-2 (C2, zero-padded to 101 cols)
    lowpass conv-as-matmul weights: y[t] = sum_l h[l] * patch[2t + l - 6].
    """
    h = DB4_H.astype(np.float64)
    C1 = np.zeros((NP_, LEN1))
    for t in range(LEN1):
        for l in range(8):
            k = 2 * t + l - 6
            if 0 <= k < NP_:
                C1[k, t] = h[l]
    M2 = np.zeros((LEN1, LEN2))
    for s in range(LEN2):
        for l in range(8):
            j = 2 * s + l - 6
            if 0 <= j < LEN1:
                M2[j, s] = h[l]
    C2 = np.zeros((NP_, LEN1))
    C2[:, :LEN2] = C1 @ M2
    out = []
    for C in (C1, C2):
        Cx = np.zeros((N_TOK, OUT_TOK))
        Cx[0, 0] = 1.0
        Cx[1:, 1:] = C
        out.append(Cx)
    return out


def _weight_pair():
    """(2, 128, 256) f32: per variant, [k, chunkA | chunkB] with M padded.

    Chunk A holds Cx rows 0..128 (tokens 0..127); chunk B holds rows for
    tokens 69..196 with the first 59 rows zeroed (they belong to chunk A).
    Output columns 102..127 are zero -> padded out rows are exact zeros.
    """
    Cx1, Cx2 = _conv_mats()
    w = np.zeros((2, 128, W_COLS), dtype=np.float32)
    for v, Cx in enumerate((Cx1, Cx2)):
        w[v, :, 0:OUT_TOK] = Cx[0:KA]
        w[v, 59:128, M_PAD:M_PAD + OUT_TOK] = Cx[KA:N_TOK]
    return w


def _level2_host(cam: np.ndarray) -> np.ndarray:
    """Replicate reference compute_gini bit-exactly with jax on CPU."""
    import jax
    import jax.numpy as jnp

    cpu = jax.devices("cpu")[0]
    with jax.default_device(cpu):
        probs = jnp.asarray(cam)
        n = probs.shape[1]
        sp = jnp.sort(probs, axis=1)
        idx = jnp.arange(1, n + 1, dtype=sp.dtype)
        gini = 2.0 * (idx * sp).sum(axis=1) / (n * sp.sum(axis=1) + 1e-8) - (n + 1) / n
        return np.asarray(gini > SOGLIA)


def _build_nc():
    nc = bacc.Bacc(None, target_bir_lowering=False, debug=False)
    f32 = mybir.dt.float32
    bf16 = mybir.dt.bfloat16
    xw = nc.declare_dram_parameter("xw", [128, B_LOC, XW_COLS], bf16, isOutput=False)
    w = nc.declare_dram_parameter("w", [128, B_LOC, W_COLS], bf16, isOutput=False)
    out = nc.declare_dram_parameter("out", [128, B_LOC, D], bf16, isOutput=True)

    with TileContext(nc) as tc:
        with (
            tc.tile_pool(name="sb", bufs=4) as pool,
            tc.tile_pool(name="ps", bufs=4, space="PSUM") as pp,
        ):
            for g in range(B_LOC // G):
                b0 = g * G
                xt = pool.tile([128, G * XW_COLS], bf16, tag="xt")
                wt = pool.tile([128, G * W_COLS], bf16, tag="wt")
                in_eng = nc.sync if g % 2 == 0 else nc.scalar
                out_eng = nc.scalar if g % 2 == 0 else nc.sync
                in_eng.dma_start(xt[:], xw[:, b0:b0 + G, :])
                out_eng.dma_start(wt[:], w[:, b0:b0 + G, :])
                ot = pool.tile([128, G * D], bf16, tag="ot")
                for j in range(G):
                    xo = j * XW_COLS
                    wo = j * W_COLS
                    ps = pp.tile([M_PAD, D], f32, tag="ps")
                    for c0, cn in ((0, 512), (512, 256)):  # PSUM-bank-aligned
                        nc.tensor.matmul(ps[:, c0:c0 + cn],
                                         wt[:, wo:wo + M_PAD],
                                         xt[:, xo + c0:xo + c0 + cn],
                                         start=True, stop=False)
                        nc.tensor.matmul(ps[:, c0:c0 + cn],
                                         wt[:, wo + M_PAD:wo + W_COLS],
                                         xt[:, xo + D + c0:xo + D + c0 + cn],
                                         start=False, stop=True)
                    nc.vector.tensor_copy(ot[:, j * D:(j + 1) * D], ps[:])
                out_eng.dma_start(out[:, b0:b0 + G, :], ot[:])
    nc.compile()
    return nc


def kernel(x: np.ndarray, cls_attention_map: np.ndarray):
    global LAST_RESULT
    import ml_dtypes

    bf16 = ml_dtypes.bfloat16
    x = np.ascontiguousarray(x, dtype=np.float32)
    cam = np.ascontiguousarray(cls_attention_map, dtype=np.float32)

    level2 = _level2_host(cam)                       # (B,) bool
    w_all = _weight_pair().astype(bf16)[level2.astype(np.int64)]  # (B,128,256)
    # interleaved input, bf16: [b, k, chunkA | chunkB]
    xwb = np.concatenate([x[:, 0:KA, :], x[:, KB_OFF:N_TOK, :]], axis=2).astype(bf16)

    if not _NC_CACHE:
        _NC_CACHE.append(_build_nc())
    nc = _NC_CACHE[0]
    in_maps = []
    for i in range(N_CORES):
        sl = slice(i * B_LOC, (i + 1) * B_LOC)
        # device wants partition-major (k, b, cols)
        in_maps.append({
            "xw": np.ascontiguousarray(xwb[sl].transpose(1, 0, 2)),
            "w": np.ascontiguousarray(w_all[sl].transpose(1, 0, 2)),
        })

    def _run():
        return run_bass_kernel_spmd(
            nc, in_maps, core_ids=list(range(N_CORES)), **RUN_KWARGS
        )

    try:
        LAST_RESULT = _run()
    except Exception:
        LAST_RESULT = _run()                         # transient NRT wedge retry

    final_x = np.concatenate(
        [r["out"].transpose(1, 0, 2)[:, :OUT_TOK, :].astype(np.float32)
         for r in LAST_RESULT.results],
        axis=0,
    )

    out_len = np.where(level2, LEN2, LEN1)
    mask = np.arange(LEN1)[None, :] < out_len[:, None]
    mask = np.concatenate([np.ones((B, 1), dtype=bool), mask], axis=1)
    return final_x, mask


# revision 34
# speedup vs baseline: 1.1056x; 1.1056x over previous
"""AdaptivePruner Trainium2 kernel: gini-routed 1/2-level db4 DWT lowpass.

Strategy
--------
- Routing (gini > SOGLIA) is computed on host with jax-on-CPU, bit-matching
  the reference's float32 arithmetic (one row's gini sits 4e-7 from the
  threshold, so arithmetic-order fidelity matters).
- The DWT is one dense matmul per sample: out[b] = Cx_b.T @ x[b] where
  Cx_b (197x102) is the host-selected composite matrix (level-1 conv, or
  level-1 o level-2 conv zero-padded, with an identity entry passing the
  cls token through). Mask is reconstructed on host.
- Pure data parallelism: batch 256 -> 8 NeuronCores x 32 samples.
- DMA shape rules (measured on this part): a dma_start fans across all 16
  SDMA engines only at 128 partitions, and per-engine throughput is
  bounded by max(bytes/25.6 GB/s, ~118 ns/descriptor). All DRAM tensors
  are laid out partition-major (k, b, ...) and moved in 4-batch groups:
  128 descriptors of 4x-size per group.
- Contraction split 128+128 with a 59-token overlap (overlap rows zeroed
  in the weight chunk B); host interleaves both chunks per partition row.
- x/weights/output all ride bf16 (host converts and upcasts): full PE
  rate, fast weight load, half the DMA bytes. M is padded to 128 so the
  padded output rows are exact zeros and every DMA is 128-partition.
- DMA issue alternates between the two HWDGE rings (SP and Activation)
  per group; PSUM->SBUF cast-copies alternate Vector/Scalar engines.
"""

import os
import sys

import numpy as np

for _p in ("/opt/trn_rl_repo", "/root/.axon_site/_ro/trn_rl_repo"):
    if os.path.isdir(_p) and _p not in sys.path:
        sys.path.append(_p)

import concourse.bacc as bacc
import concourse.mybir as mybir
from concourse.tile import TileContext
from concourse.bass_utils import run_bass_kernel_spmd

SOGLIA = 0.333
DB4_H = np.array([0.23037781330885523, 0.7148465705525415, 0.6308807679295904,
                  -0.02798376941698385, -0.18703481171888114, 0.030841381835986965,
                  0.032883011666982945, -0.010597401784997278], dtype=np.float32)

B, N_TOK, D = 256, 197, 768          # x: (B, 197, 768)
NP_ = 196                            # patch tokens
LEN1, LEN2 = 101, 54                 # dwt output lengths
OUT_TOK = LEN1 + 1                   # 102 = cls + padded patches
M_PAD = 128                          # matmul M / padded output rows
N_CORES = 8
B_LOC = 32                           # batches per core
G = 4                                # batches per DMA group
KA = 128                             # chunk A: tokens 0..128
KB_OFF = N_TOK - 128                 # 69: chunk B covers tokens 69..197
XW_COLS = 2 * D                      # interleaved input row: [chunk A | chunk B]
W_COLS = 2 * M_PAD                   # merged weight row: [chunk A | chunk B]
LAST_RESULT = None                   # BassKernelResults of the last run
RUN_KWARGS = {}                      # extra kwargs for run_bass_kernel_spmd (test harness)
_NC_CACHE = []                       # compiled Bacc program, built once per process


def _conv_mats():
    """Composite DWT matrices in float64: Cx1/Cx2 (197, 102).

    Row 0 / col 0 pass the cls token through; rows 1+k / cols 1+t hold the
    level-1 (C1) or level-1 compose level-2 (C2, zero-padded to 101 cols)
    lowpass conv-as-matmul weights: y[t] = sum_l h[l] * patch[2t + l - 6].
    """
    h = DB4_H.astype(np.float64)
    C1 = np.zeros((NP_, LEN1))
    for t in range(LEN1):
        for l in range(8):
            k = 2 * t + l - 6
            if 0 <= k < NP_:
                C1[k, t] = h[l]
    M2 = np.zeros((LEN1, LEN2))
    for s in range(LEN2):
        for l in range(8):
            j = 2 * s + l - 6
            if 0 <= j < LEN1:
                M2[j, s] = h[l]
    C2 = np.zeros((NP_, LEN1))
    C2[:, :LEN2] = C1 @ M2
    out = []
    for C in (C1, C2):
        Cx = np.zeros((N_TOK, OUT_TOK))
        Cx[0, 0] = 1.0
        Cx[1:, 1:] = C
        out.append(Cx)
    return out


def _weight_pair():
    """(2, 128, 256) f32: per variant, [k, chunkA | chunkB] with M padded.

    Chunk A holds Cx rows 0..128 (tokens 0..127); chunk B holds rows for
    tokens 69..196 with the first 59 rows zeroed (they belong to chunk A).
    Output columns 102..127 are zero -> padded out rows are exact zeros.
    """
    Cx1, Cx2 = _conv_mats()
    w = np.zeros((2, 128, W_COLS), dtype=np.float32)
    for v, Cx in enumerate((Cx1, Cx2)):
        w[v, :, 0:OUT_TOK] = Cx[0:KA]
        w[v, 59:128, M_PAD:M_PAD + OUT_TOK] = Cx[KA:N_TOK]
    return w


def _level2_host(cam: np.ndarray) -> np.ndarray:
    """Replicate reference compute_gini bit-exactly with jax on CPU."""
    import jax
    import jax.numpy as jnp

    cpu = jax.devices("cpu")[0]
    with jax.default_device(cpu):
        probs = jnp.asarray(cam)
        n = probs.shape[1]
        sp = jnp.sort(probs, axis=1)
        idx = jnp.arange(1, n + 1, dtype=sp.dtype)
        gini = 2.0 * (idx * sp).sum(axis=1) / (n * sp.sum(axis=1) + 1e-8) - (n + 1) / n
        return np.asarray(gini > SOGLIA)


def _build_nc():
    nc = bacc.Bacc(None, target_bir_lowering=False, debug=False)
    f32 = mybir.dt.float32
    bf16 = mybir.dt.bfloat16
    xw = nc.declare_dram_parameter("xw", [128, B_LOC, XW_COLS], bf16, isOutput=False)
    w = nc.declare_dram_parameter("w", [128, B_LOC, W_COLS], bf16, isOutput=False)
    out = nc.declare_dram_parameter("out", [128, B_LOC, D], bf16, isOutput=True)

    with TileContext(nc) as tc:
        with (
            tc.tile_pool(name="sb", bufs=4) as pool,
            tc.tile_pool(name="ps", bufs=4, space="PSUM") as pp,
        ):
            for g in range(B_LOC // G):
                b0 = g * G
                xt = pool.tile([128, G * XW_COLS], bf16, tag="xt")
                wt = pool.tile([128, G * W_COLS], bf16, tag="wt")
                in_eng = nc.sync if g % 2 == 0 else nc.scalar
                out_eng = nc.scalar if g % 2 == 0 else nc.sync
                in_eng.dma_start(xt[:], xw[:, b0:b0 + G, :])
                out_eng.dma_start(wt[:], w[:, b0:b0 + G, :])
                ot = pool.tile([128, G * D], bf16, tag="ot")
                for j in range(G):
                    xo = j * XW_COLS
                    wo = j * W_COLS
                    ps = pp.tile([M_PAD, D], f32, tag="ps")
                    for c0, cn in ((0, 512), (512, 256)):  # PSUM-bank-aligned
                        nc.tensor.matmul(ps[:, c0:c0 + cn],
                                         wt[:, wo:wo + M_PAD],
                                         xt[:, xo + c0:xo + c0 + cn],
                                         start=True, stop=False)
                        nc.tensor.matmul(ps[:, c0:c0 + cn],
                                         wt[:, wo + M_PAD:wo + W_COLS],
                                         xt[:, xo + D + c0:xo + D + c0 + cn],
                                         start=False, stop=True)
                    if j % 2 == 0:
                        nc.vector.tensor_copy(ot[:, j * D:(j + 1) * D], ps[:])
                    else:
                        nc.scalar.copy(ot[:, j * D:(j + 1) * D], ps[:])
                out_eng.dma_start(out[:, b0:b0 + G, :], ot[:])
    nc.compile()
    return nc


def kernel(x: np.ndarray, cls_attention_map: np.ndarray):
    global LAST_RESULT
    import ml_dtypes

    bf16 = ml_dtypes.bfloat16
    x = np.ascontiguousarray(x, dtype=np.float32)
    cam = np.ascontiguousarray(cls_attention_map, dtype=np.float32)

    level2 = _level2_host(cam)                       # (B,) bool
    w_all = _weight_pair().astype(bf16)[level2.astype(np.int64)]  # (B,128,256)
    # interleaved input, bf16: [b, k, chunkA | chunkB]
    xwb = np.concatenate([x[:, 0:KA, :], x[:, KB_OFF:N_TOK, :]], axis=2).astype(bf16)

    if not _NC_CACHE:
        _NC_CACHE.append(_build_nc())
    nc = _NC_CACHE[0]
    in_maps = []
    for i in range(N_CORES):
        sl = slice(i * B_LOC, (i + 1) * B_LOC)
        # device wants partition-major (k, b, cols)
        in_maps.append({
            "xw": np.ascontiguousarray(xwb[sl].transpose(1, 0, 2)),
            "w": np.ascontiguousarray(w_all[sl].transpose(1, 0, 2)),
        })

    def _run():
        return run_bass_kernel_spmd(
            nc, in_maps, core_ids=list(range(N_CORES)), **RUN_KWARGS
        )

    try:
        LAST_RESULT = _run()
    except Exception:
        LAST_RESULT = _run()                         # transient NRT wedge retry

    final_x = np.concatenate(
        [r["out"].transpose(1, 0, 2)[:, :OUT_TOK, :].astype(np.float32)
         for r in LAST_RESULT.results],
        axis=0,
    )

    out_len = np.where(level2, LEN2, LEN1)
    mask = np.arange(LEN1)[None, :] < out_len[:, None]
    mask = np.concatenate([np.ones((B, 1), dtype=bool), mask], axis=1)
    return final_x, mask


# revision 36
# speedup vs baseline: 1.1373x; 1.0287x over previous
"""AdaptivePruner Trainium2 kernel: gini-routed 1/2-level db4 DWT lowpass.

Strategy
--------
- Routing (gini > SOGLIA) is computed on host with jax-on-CPU, bit-matching
  the reference's float32 arithmetic (one row's gini sits 4e-7 from the
  threshold, so arithmetic-order fidelity matters).
- The DWT is one dense matmul per sample: out[b] = Cx_b.T @ x[b] where
  Cx_b (197x102) is the host-selected composite matrix (level-1 conv, or
  level-1 o level-2 conv zero-padded, with an identity entry passing the
  cls token through). Mask is reconstructed on host.
- Pure data parallelism: batch 256 -> 8 NeuronCores x 32 samples.
- DMA shape rules (measured on this part): a dma_start fans across all 16
  SDMA engines only at 128 partitions, and per-engine throughput is
  bounded by max(bytes/25.6 GB/s, ~118 ns/descriptor). All DRAM tensors
  are laid out partition-major (k, b, ...) and moved in 4-batch groups:
  128 descriptors of 4x-size per group.
- Contraction split 128+128 with a 59-token overlap (overlap rows zeroed
  in the weight chunk B); host interleaves both chunks per partition row.
- x/weights/output all ride bf16 (host converts and upcasts): full PE
  rate, fast weight load, half the DMA bytes. M is padded to 128 so the
  padded output rows are exact zeros and every DMA is 128-partition.
- DMA issue alternates between the two HWDGE rings (SP and Activation)
  per group; PSUM->SBUF cast-copies alternate Vector/Scalar engines.
"""

import os
import sys

import numpy as np

for _p in ("/opt/trn_rl_repo", "/root/.axon_site/_ro/trn_rl_repo"):
    if os.path.isdir(_p) and _p not in sys.path:
        sys.path.append(_p)

import concourse.bacc as bacc
import concourse.mybir as mybir
from concourse.tile import TileContext
from concourse.bass_utils import run_bass_kernel_spmd

SOGLIA = 0.333
DB4_H = np.array([0.23037781330885523, 0.7148465705525415, 0.6308807679295904,
                  -0.02798376941698385, -0.18703481171888114, 0.030841381835986965,
                  0.032883011666982945, -0.010597401784997278], dtype=np.float32)

B, N_TOK, D = 256, 197, 768          # x: (B, 197, 768)
NP_ = 196                            # patch tokens
LEN1, LEN2 = 101, 54                 # dwt output lengths
OUT_TOK = LEN1 + 1                   # 102 = cls + padded patches
M_PAD = 128                          # matmul M / padded output rows
N_CORES = 8
B_LOC = 32                           # batches per core
G = 4                                # batches per DMA group
KA = 128                             # chunk A: tokens 0..128
KB_OFF = N_TOK - 128                 # 69: chunk B covers tokens 69..197
XW_COLS = 2 * D                      # interleaved input row: [chunk A | chunk B]
W_COLS = 2 * M_PAD                   # merged weight row: [chunk A | chunk B]
LAST_RESULT = None                   # BassKernelResults of the last run
RUN_KWARGS = {}                      # extra kwargs for run_bass_kernel_spmd (test harness)
_NC_CACHE = []                       # compiled Bacc program, built once per process


def _conv_mats():
    """Composite DWT matrices in float64: Cx1/Cx2 (197, 102).

    Row 0 / col 0 pass the cls token through; rows 1+k / cols 1+t hold the
    level-1 (C1) or level-1 compose level-2 (C2, zero-padded to 101 cols)
    lowpass conv-as-matmul weights: y[t] = sum_l h[l] * patch[2t + l - 6].
    """
    h = DB4_H.astype(np.float64)
    C1 = np.zeros((NP_, LEN1))
    for t in range(LEN1):
        for l in range(8):
            k = 2 * t + l - 6
            if 0 <= k < NP_:
                C1[k, t] = h[l]
    M2 = np.zeros((LEN1, LEN2))
    for s in range(LEN2):
        for l in range(8):
            j = 2 * s + l - 6
            if 0 <= j < LEN1:
                M2[j, s] = h[l]
    C2 = np.zeros((NP_, LEN1))
    C2[:, :LEN2] = C1 @ M2
    out = []
    for C in (C1, C2):
        Cx = np.zeros((N_TOK, OUT_TOK))
        Cx[0, 0] = 1.0
        Cx[1:, 1:] = C
        out.append(Cx)
    return out


def _weight_pair():
    """(2, 128, 256) f32: per variant, [k, chunkA | chunkB] with M padded.

    Chunk A holds Cx rows 0..128 (tokens 0..127); chunk B holds rows for
    tokens 69..196 with the first 59 rows zeroed (they belong to chunk A).
    Output columns 102..127 are zero -> padded out rows are exact zeros.
    """
    Cx1, Cx2 = _conv_mats()
    w = np.zeros((2, 128, W_COLS), dtype=np.float32)
    for v, Cx in enumerate((Cx1, Cx2)):
        w[v, :, 0:OUT_TOK] = Cx[0:KA]
        w[v, 59:128, M_PAD:M_PAD + OUT_TOK] = Cx[KA:N_TOK]
    return w


def _level2_host(cam: np.ndarray) -> np.ndarray:
    """Replicate reference compute_gini bit-exactly with jax on CPU."""
    import jax
    import jax.numpy as jnp

    cpu = jax.devices("cpu")[0]
    with jax.default_device(cpu):
        probs = jnp.asarray(cam)
        n = probs.shape[1]
        sp = jnp.sort(probs, axis=1)
        idx = jnp.arange(1, n + 1, dtype=sp.dtype)
        gini = 2.0 * (idx * sp).sum(axis=1) / (n * sp.sum(axis=1) + 1e-8) - (n + 1) / n
        return np.asarray(gini > SOGLIA)


def _build_nc():
    nc = bacc.Bacc(None, target_bir_lowering=False, debug=False)
    f32 = mybir.dt.float32
    bf16 = mybir.dt.bfloat16
    xw = nc.declare_dram_parameter("xw", [128, B_LOC, XW_COLS], bf16, isOutput=False)
    w = nc.declare_dram_parameter("w", [128, B_LOC, W_COLS], bf16, isOutput=False)
    out = nc.declare_dram_parameter("out", [128, B_LOC, D], bf16, isOutput=True)

    with TileContext(nc) as tc:
        with (
            tc.tile_pool(name="sb", bufs=4) as pool,
            tc.tile_pool(name="ps", bufs=4, space="PSUM") as pp,
        ):
            # HAM warm-up: keep PE busy during the preamble/first DMA fill so
            # the first real matmuls start at full clock (~3.4 us of activity
            # flips the PE clock gate from 1.2 to 2.4 GHz).
            wu = pool.tile([128, 512], bf16, tag="wu")
            nc.gpsimd.memset(wu[:], 0.0)
            pw = pp.tile([M_PAD, 512], f32, tag="ps")
            for _ in range(8):
                nc.tensor.matmul(pw[:], wu[:, 0:128], wu[:], start=True, stop=True)
            for g in range(B_LOC // G):
                b0 = g * G
                xt = pool.tile([128, G * XW_COLS], bf16, tag="xt")
                wt = pool.tile([128, G * W_COLS], bf16, tag="wt")
                in_eng = nc.sync if g % 2 == 0 else nc.scalar
                out_eng = nc.scalar if g % 2 == 0 else nc.sync
                in_eng.dma_start(xt[:], xw[:, b0:b0 + G, :])
                out_eng.dma_start(wt[:], w[:, b0:b0 + G, :])
                ot = pool.tile([128, G * D], bf16, tag="ot")
                for j in range(G):
                    xo = j * XW_COLS
                    wo = j * W_COLS
                    ps = pp.tile([M_PAD, D], f32, tag="ps")
                    for c0, cn in ((0, 512), (512, 256)):  # PSUM-bank-aligned
                        nc.tensor.matmul(ps[:, c0:c0 + cn],
                                         wt[:, wo:wo + M_PAD],
                                         xt[:, xo + c0:xo + c0 + cn],
                                         start=True, stop=False)
                        nc.tensor.matmul(ps[:, c0:c0 + cn],
                                         wt[:, wo + M_PAD:wo + W_COLS],
                                         xt[:, xo + D + c0:xo + D + c0 + cn],
                                         start=False, stop=True)
                    if j % 2 == 0:
                        nc.vector.tensor_copy(ot[:, j * D:(j + 1) * D], ps[:])
                    else:
                        nc.scalar.copy(ot[:, j * D:(j + 1) * D], ps[:])
                out_eng.dma_start(out[:, b0:b0 + G, :], ot[:])
    nc.compile()
    return nc


def kernel(x: np.ndarray, cls_attention_map: np.ndarray):
    global LAST_RESULT
    import ml_dtypes

    bf16 = ml_dtypes.bfloat16
    x = np.ascontiguousarray(x, dtype=np.float32)
    cam = np.ascontiguousarray(cls_attention_map, dtype=np.float32)

    level2 = _level2_host(cam)                       # (B,) bool
    w_all = _weight_pair().astype(bf16)[level2.astype(np.int64)]  # (B,128,256)
    # interleaved input, bf16: [b, k, chunkA | chunkB]
    xwb = np.concatenate([x[:, 0:KA, :], x[:, KB_OFF:N_TOK, :]], axis=2).astype(bf16)

    if not _NC_CACHE:
        _NC_CACHE.append(_build_nc())
    nc = _NC_CACHE[0]
    in_maps = []
    for i in range(N_CORES):
        sl = slice(i * B_LOC, (i + 1) * B_LOC)
        # device wants partition-major (k, b, cols)
        in_maps.append({
            "xw": np.ascontiguousarray(xwb[sl].transpose(1, 0, 2)),
            "w": np.ascontiguousarray(w_all[sl].transpose(1, 0, 2)),
        })

    def _run():
        return run_bass_kernel_spmd(
            nc, in_maps, core_ids=list(range(N_CORES)), **RUN_KWARGS
        )

    try:
        LAST_RESULT = _run()
    except Exception:
        LAST_RESULT = _run()                         # transient NRT wedge retry

    final_x = np.concatenate(
        [r["out"].transpose(1, 0, 2)[:, :OUT_TOK, :].astype(np.float32)
         for r in LAST_RESULT.results],
        axis=0,
    )

    out_len = np.where(level2, LEN2, LEN1)
    mask = np.arange(LEN1)[None, :] < out_len[:, None]
    mask = np.concatenate([np.ones((B, 1), dtype=bool), mask], axis=1)
    return final_x, mask
